# revision 14
# baseline (speedup 1.0000x reference)
"""Trainium2 kernel for nn_Direction: out = input @ Q.T, Q from QR(weight + 1e-8).

Strategy:
  - Host: QR of the small 512x512 weight (fp32), then pre-tile each batch
    shard to the exact SBUF layout ([n_iter, 128, KT, bt], contraction dim
    on partitions) so every device DMA is fully contiguous per partition
    (8 KB/partition lines at bt=1024).
  - Device (8 cores, data-parallel over batch): single-pass fp16 tiled
    matmul. The correctness gate is rel_err < 2e-2; fp16 quantization of
    both operands gives ~4e-4, so the old 3-pass hi/lo scheme (3x tensor
    work) is unnecessary. Per 128-batch tile: 4 matmuls (K=512) accumulate
    in one PSUM bank, evicted fp32->fp16 on alternating DVE/Act engines
    into a per-iteration staging tile, one batched output DMA per
    iteration. fp16 output halves the out-DMA bytes, keeping DMA (~99us
    measured in isolation) under the PE streaming roofline (109us/core).
  - Gather: concatenate the 8 output shards, cast fp16 -> fp32 on host.

Measured on HW (differential bench, For_i rep loop, [129,513]):
  rel err 3.6e-4; ~150us/rep at bt=1024 vs 418us for the 3-pass baseline.
  Isolated subsystems match the TimelineSim cost model (DMA-only 99us,
  PE+evict 129us); the remaining ~20us is DMA/PE/evict concurrency cost
  that no tested knob (bt, bufs, evict engines, psum depth, k-sliced or
  split DMAs, out-row interleave, q duplication) removed.

Modes: fp16 (default, single pass, fp16 out), fp16o32 (fp16 out fp32),
bf16, fp32, f32r, fp16x2 (legacy 3-pass hi/lo, fp32-accurate), plus
timing diagnostics (dmaonly/peonly/noevict).
"""

import numpy as np

import concourse.bacc as bacc
import concourse.mybir as mybir
import concourse.tile as tile
from concourse.bass_utils import run_bass_kernel_spmd

B_FULL = 131072
D = 512
N_CORES = 8
B_LOC = B_FULL // N_CORES  # 16384
P = 128
BT = 512  # batch rows per loop iteration
KT = D // P  # 4 k-tiles
SB = BT // P  # 4 psum sub-tiles per iteration

MODE = "fp16"
# best measured config (HW differential bench): bt=1024 batch rows/iteration
BEST_BT = 1024
BEST_OF = 1

# mode -> (in dtype, out dtype, passes)
_DT = {
    "fp16": (mybir.dt.float16, mybir.dt.float16, [("a0", "q0")]),
    "fp16o32": (mybir.dt.float16, mybir.dt.float32, [("a0", "q0")]),
    "bf16": (mybir.dt.bfloat16, mybir.dt.float16, [("a0", "q0")]),
    "fp32": (mybir.dt.float32, mybir.dt.float32, [("a0", "q0")]),
    "f32r": (mybir.dt.float32r, mybir.dt.float32, [("a0", "q0")]),
    "fp16x2": (
        mybir.dt.float16,
        mybir.dt.float32,
        [("a0", "q0"), ("a1", "q0"), ("a0", "q1")],
    ),
    # timing diagnostics (not numerically meaningful outputs):
    "dmaonly": (mybir.dt.float16, mybir.dt.float16, [("a0", "q0")]),
    "peonly": (mybir.dt.float16, mybir.dt.float16, [("a0", "q0")]),
    "noevict": (mybir.dt.float16, mybir.dt.float16, [("a0", "q0")]),
}

_NP_DT = {
    mybir.dt.float16: np.float16,
    mybir.dt.float32: np.float32,
    mybir.dt.float32r: np.float32,
}

_CACHE = {}


def _np_in_dt(dt_in):
    if dt_in == mybir.dt.bfloat16:
        import ml_dtypes

        return ml_dtypes.bfloat16
    return _NP_DT[dt_in]


def _build(mode, b_loc, reps=1, dynamic=False, bt=BT, ain_bufs=3, aout_bufs=3,
           ps_bufs=8, evict="alt", ps2=False, kdma=False, qdup=1, of=1, og=1):
    dt_in, dt_out, passes = _DT[mode]
    a_names = sorted({a for a, _ in passes})
    q_names = sorted({q for _, q in passes})
    n_iter = b_loc // bt
    sb_n = bt // P

    nc = bacc.Bacc("TRN2", target_bir_lowering=False, debug=False)
    # pre-tiled on host: a[it, p, k, b'] = A.T[k*128+p, it*bt+b']
    a_dram = {
        n: nc.dram_tensor(n, [n_iter, P, KT, bt], dt_in, kind="ExternalInput").ap()
        for n in a_names
    }
    # pre-tiled on host: q[p, k, n] = Q[n, k*128+p]
    q_dram = {
        n: nc.dram_tensor(n, [P, KT, D], dt_in, kind="ExternalInput").ap()
        for n in q_names
    }
    out_dram = nc.dram_tensor(
        "out", [b_loc, D], dt_out, kind="ExternalOutput"
    ).ap()
    if of == 1:
        # out rows b = it*bt + j*128 + p  <->  out_r[it, p, j, n]
        out_r = out_dram.rearrange("(i j p) n -> i p j n", p=P, j=sb_n)
    else:
        # host permutes stationary columns so psum group (blk, j) holds DRAM
        # rows it*bt + blk*128*of + p*of + j -> partition p writes `of`
        # consecutive rows = of*1KB contiguous runs per partition.
        out_r = out_dram.rearrange(
            "(i blk p j) n -> i p blk (j n)", p=P, j=of, blk=sb_n // of
        )

    with tile.TileContext(nc) as tc:
        with (
            tc.tile_pool(name="consts", bufs=1) as consts,
            tc.tile_pool(name="ain", bufs=ain_bufs) as ain,
            tc.tile_pool(name="aout", bufs=aout_bufs) as aout,
            tc.tile_pool(name="ps", bufs=ps_bufs, space="PSUM") as ps_pool,
        ):
            do_in = mode != "peonly"
            do_mm = mode != "dmaonly"
            do_evict = mode not in ("dmaonly", "noevict")
            do_out = mode != "peonly"

            q_tiles = {}
            for qn in q_names:
                copies = []
                for c in range(qdup):
                    qt = consts.tile([P, KT, D], dt_in, name=f"qt_{qn}_{c}")
                    nc.sync.dma_start(out=qt[:, :, :], in_=q_dram[qn])
                    copies.append(qt)
                q_tiles[qn] = copies
            a_res = None
            if not do_in:
                a_res = consts.tile([P, KT, bt], dt_in, name="a_res")
                nc.sync.dma_start(out=a_res[:, :, :], in_=a_dram["a0"][0])
            o_res = None
            if not do_evict and do_out:
                o_res = consts.tile([P, sb_n, D], dt_out, name="o_res")
                nc.vector.memset(o_res[:, :, :], 0.0)

            def do_evict_copy(dst, src, sb):
                if evict == "vector":
                    nc.vector.tensor_copy(dst, src)
                elif evict == "any":
                    nc.any.tensor_copy(dst, src)
                elif evict == "split":
                    h = dst.shape[-1] // 2
                    nc.vector.tensor_copy(dst[..., :h], src[..., :h])
                    nc.scalar.activation(
                        dst[..., h:], src[..., h:],
                        mybir.ActivationFunctionType.Copy,
                    )
                else:  # alt
                    if sb % 2 == 0:
                        nc.vector.tensor_copy(dst, src)
                    else:
                        nc.scalar.activation(
                            dst, src, mybir.ActivationFunctionType.Copy
                        )

            def body():
                for it in range(n_iter):
                    a_tiles = {}
                    for an in a_names:
                        if not do_in:
                            a_tiles[an] = a_res
                            continue
                        at = ain.tile(
                            [P, KT, bt], dt_in, name=f"at_{an}", tag=f"at_{an}"
                        )
                        if kdma:
                            for k in range(KT):
                                nc.sync.dma_start(
                                    out=at[:, k, :], in_=a_dram[an][it, :, k]
                                )
                        else:
                            nc.sync.dma_start(out=at[:, :, :], in_=a_dram[an][it])
                        a_tiles[an] = at
                    if do_evict:
                        if of == 1:
                            ot = aout.tile([P, sb_n, D], dt_out, name="ot", tag="ot")
                        else:
                            ot = aout.tile(
                                [P, sb_n // of, of * D], dt_out, name="ot", tag="ot"
                            )
                    else:
                        ot = o_res

                    def ot_dst(sb):
                        if of == 1:
                            return ot[:, sb, :]
                        return ot[:, sb // of, (sb % of) * D : (sb % of + 1) * D]

                    def ot_dst2(sb):
                        if of == 1:
                            return ot[:, sb - 1 : sb + 1, :]
                        assert sb % of >= 1
                        return ot[:, sb // of, (sb % of - 1) * D : (sb % of + 1) * D]
                    ps = None
                    for sb in range(sb_n):
                        if not do_mm:
                            continue
                        if ps2:
                            if sb % 2 == 0:
                                ps = ps_pool.tile(
                                    [P, 2, D], mybir.dt.float32, name="ps", tag="ps"
                                )
                            psv = ps[:, sb % 2, :]
                        else:
                            ps = ps_pool.tile(
                                [P, D], mybir.dt.float32, name="ps", tag="ps"
                            )
                            psv = ps[:, :]
                        n_mm = len(passes) * KT
                        mm = 0
                        for an, qn in passes:
                            at = a_tiles[an]
                            qt = q_tiles[qn][sb % qdup if qdup > 1 else 0]
                            for k in range(KT):
                                nc.tensor.matmul(
                                    psv,
                                    at[:, k, sb * P : (sb + 1) * P],
                                    qt[:, k, :],
                                    start=(mm == 0),
                                    stop=(mm == n_mm - 1),
                                )
                                mm += 1
                        if not do_evict:
                            continue
                        if ps2:
                            if sb % 2 == 1:
                                do_evict_copy(ot_dst2(sb), ps[:, :, :], sb)
                        else:
                            do_evict_copy(ot_dst(sb), psv, sb)
                    if do_out:
                        if og == 1:
                            nc.sync.dma_start(out=out_r[it], in_=ot[:, :, :])
                        else:
                            g = ot.shape[1] // og
                            for o in range(og):
                                nc.sync.dma_start(
                                    out=out_r[it][:, o * g : (o + 1) * g],
                                    in_=ot[:, o * g : (o + 1) * g, :],
                                )

            if dynamic and reps > 1:
                with tc.For_i(0, reps, 1):
                    body()
            else:
                for _ in range(reps):
                    body()

    nc.compile()
    return nc


def _get_nc(mode, b_loc, **kw):
    return _get_nc_reps(mode, b_loc, 1, **kw)


def _get_nc_reps(mode, b_loc, reps, dynamic=False, **kw):
    key = (mode, b_loc, reps, dynamic, tuple(sorted(kw.items())))
    if key not in _CACHE:
        _CACHE[key] = _build(mode, b_loc, reps, dynamic, **kw)
    return _CACHE[key]


def _split16(x):
    hi = x.astype(np.float16)
    lo = (x - hi.astype(np.float32)).astype(np.float16)
    return hi, lo


def _tile_a(at_np, b_loc, bt, of=1):
    """A.T shard (D, b_loc) -> [n_iter, P, KT, bt] with
    a[it, p, k, b'] = A.T[k*128+p, it*bt+b']. For of>1, batch columns are
    interleaved within 128*of blocks (c = blk*128*of + j*128 + m <-> batch
    row blk*128*of + m*of + j) so each psum partition owns `of` consecutive
    output rows."""
    n_iter = b_loc // bt
    if of == 1:
        return np.ascontiguousarray(
            at_np.reshape(KT, P, n_iter, bt).transpose(2, 1, 0, 3)
        )
    nblk = bt // (P * of)
    return np.ascontiguousarray(
        at_np.reshape(KT, P, n_iter, nblk, P, of)
        .transpose(2, 1, 0, 3, 5, 4)
        .reshape(n_iter, P, KT, bt)
    )


def _tile_q(qt_np):
    """Q.T (D, D) -> [P, KT, D] with q[p, k, n] = Q.T[k*128+p, n]."""
    return np.ascontiguousarray(qt_np.reshape(KT, P, D).transpose(1, 0, 2))


def _prep_inputs(mode, input_np, qt_np, n_cores, b_loc, bt=BT, of=1):
    """Build per-core input maps. input_np: (n_cores*b_loc, D) fp32 row-major.
    qt_np: (D, D) fp32, qt_np[m, n] = Q[n, m]."""
    dt_in, _, passes = _DT[mode]
    maps = []
    if mode == "fp16x2":
        qh, ql = _split16(qt_np)
        qh, ql = _tile_q(qh), _tile_q(ql)
        for i in range(n_cores):
            at = np.ascontiguousarray(input_np[i * b_loc : (i + 1) * b_loc].T)
            ah, al = _split16(at)
            maps.append(
                {
                    "a0": _tile_a(ah, b_loc, bt, of),
                    "a1": _tile_a(al, b_loc, bt, of),
                    "q0": qh,
                    "q1": ql,
                }
            )
    else:
        cast_dt = _np_in_dt(dt_in)
        q0 = _tile_q(qt_np.astype(cast_dt))
        # cast before transposing: half the bytes moved for 16-bit modes
        inp_c = input_np.astype(cast_dt, copy=False)
        for i in range(n_cores):
            at = np.ascontiguousarray(inp_c[i * b_loc : (i + 1) * b_loc].T)
            maps.append({"a0": _tile_a(at, b_loc, bt, of), "q0": q0})
    return maps


def _compute_qt(weight_np):
    """Q from QR(weight + 1e-8), transposed. Prefer jax-on-CPU so Q matches the
    fp32 jax reference bit-for-bit when possible; fall back to LAPACK (both are
    Householder QR and agree to ~1e-6, so either is well within tolerance)."""
    w = weight_np.astype(np.float32)
    try:
        import jax
        import jax.numpy as jnp

        cpu = jax.devices("cpu")[0]
        with jax.default_device(cpu):
            q, _ = jnp.linalg.qr(jax.device_put(w, cpu) + 1e-8)
        q = np.asarray(q)
    except Exception:
        q, _ = np.linalg.qr(w + np.float32(1e-8))
    return np.ascontiguousarray(q.T.astype(np.float32))


def run(input_np, weight_np, mode=None, n_cores=N_CORES, b_loc=None, bt=BT,
        of=1, nc_kw=None, **run_kwargs):
    mode = mode or MODE
    b_loc = b_loc or (input_np.shape[0] // n_cores)
    assert input_np.shape[0] == n_cores * b_loc, (
        f"batch {input_np.shape[0]} not divisible into {n_cores} cores"
    )
    assert b_loc % bt == 0 and input_np.shape[1] == D

    qt = _compute_qt(weight_np)

    nc = _get_nc(mode, b_loc, bt=bt, of=of, **(nc_kw or {}))
    in_maps = _prep_inputs(mode, np.asarray(input_np), qt, n_cores, b_loc, bt=bt,
                           of=of)
    res = run_bass_kernel_spmd(nc, in_maps, list(range(n_cores)), **run_kwargs)
    out = np.concatenate([res.results[i]["out"] for i in range(n_cores)], axis=0)
    return out, res


def kernel(input, weight):
    out, _ = run(
        np.asarray(input, dtype=np.float32),
        np.asarray(weight, dtype=np.float32),
        bt=BEST_BT,
        of=BEST_OF,
    )
    return np.ascontiguousarray(out.astype(np.float32))
